# revision 1
# baseline (speedup 1.0000x reference)
"""Trainium2 Bass kernel for nn_DecentralizedCoordinator (GNN message passing).

Strategy (8 NeuronCores, SPMD):
- Nodes sharded by id: core k owns rows [k*12544, (k+1)*12544) (98 blocks of 128).
- Edges partitioned by destination core/block and source subtable (4 x 25088
  rows, int16 dma_gather indices). Per-(block,subtable) slot counts are the max
  over cores (shared SPMD program); slots pack back-to-back into 128-row
  columns, gathered f32 on-device with dma_gather.
- segment_sum via one-hot matmul per (column, block) occurrence:
  PSUM[d,f] += one_hot(dst_local)^T @ G_col; counts via ones-column matmul.
- mean = sums * (1/max(cnt,1)); MLP transposed on-chip:
  meanT -> w1 -> gelu(+b1) -> (lhsT=hT, rhs=w2) + b2 -> reports (node-major).
- Leader election (launch 2): host reshapes device logits into a per-dst padded
  layout [dst_local=partition, slot]; device does reduce_max / is_equal /
  mult(src+1) / reduce_max (exact reference tie-break semantics).
- Final gather (launch 3): reports[leader] via dma_gather over 4 subtables.

Host only shards/reshapes/gathers-by-index between launches; all arithmetic on
values happens on device.
"""
import os
import sys
import hashlib

import numpy as np
import ml_dtypes

sys.path.insert(0, "/opt/trn_rl_repo")

import concourse.bass as bass
import concourse.tile as tile
from concourse import bacc, mybir
from concourse.bass_utils import run_bass_kernel_spmd
from concourse.masks import make_identity

dt = mybir.dt
bf16 = ml_dtypes.bfloat16

P = 128
NCORES = 8
BPC = 98                 # dst blocks per core
NPC = BPC * P            # 12544 nodes per core
NPAD = NCORES * NPC      # 100352 padded node count
NSUB = 4
SUB = NPAD // NSUB       # 25088 rows per gather subtable
H = 128
C = 128
CW = 32                  # gather-window width (columns of 128 rows)
NEG = -3.0e38

CORES = list(range(NCORES))


def _wrap_idx16(local_idx):
    """[NI] int array -> [128, NI//16] int16 (16-wrap, replicated x8)."""
    ni = len(local_idx)
    assert ni % 16 == 0
    w = np.asarray(local_idx, np.int16).reshape(ni // 16, 16).T  # [16, NI/16]
    return np.tile(w, (8, 1)).copy()


def _assign_nodes(col, sub, n_nodes):
    """Balanced node -> (core, block, slot): equalize per-(b,t) in-degree sums
    across cores. Returns node2kbp [N,3] and inv [NCORES,BPC,P] (orig id, -1)."""
    indeg4 = np.zeros((n_nodes, NSUB), np.int64)
    np.add.at(indeg4, (col, sub), 1)
    indeg = indeg4.sum(axis=1)
    order = np.argsort(-indeg, kind="stable")
    node2kbp = np.zeros((n_nodes, 3), np.int64)
    inv = np.full((NCORES, BPC, P), -1, np.int64)
    BIG = 1 << 40
    for b in range(BPC):
        sl = order[b * NCORES * P : (b + 1) * NCORES * P]
        loads = np.zeros((NCORES, NSUB), np.float64)
        caps = np.full(NCORES, P, np.int64)
        slots = np.zeros(NCORES, np.int64)
        for nd in sl:  # vector-aware greedy: min sum of squared loads
            v = indeg4[nd]
            cost = ((loads + v) ** 2).sum(axis=1)
            cost[caps == 0] = BIG
            kbest = int(np.argmin(cost))
            loads[kbest] += v
            caps[kbest] -= 1
            p = int(slots[kbest])
            slots[kbest] += 1
            node2kbp[nd] = (kbest, b, p)
            inv[kbest, b, p] = nd
    return node2kbp, inv


def _preprocess(edge_index):
    """Chunk/occurrence structure + per-core index arrays from edge_index."""
    row = np.asarray(edge_index[0], np.int64)
    col = np.asarray(edge_index[1], np.int64)
    n_nodes = 100000

    sub = row // SUB
    loc = (row % SUB).astype(np.int64)
    node2kbp, inv = _assign_nodes(col, sub, n_nodes)
    core = node2kbp[col, 0]
    blk = node2kbp[col, 1]
    dstl = node2kbp[col, 2]

    gkey = (core * BPC + blk) * NSUB + sub
    order = np.argsort(gkey, kind="stable")
    loc_s = loc[order]
    dstl_s = dstl[order]
    counts = np.bincount(gkey, minlength=NCORES * BPC * NSUB).reshape(
        NCORES, BPC, NSUB
    )
    starts = np.concatenate([[0], np.cumsum(counts.reshape(-1))[:-1]]).reshape(
        counts.shape
    )

    # shared slot counts: per (b, t) = max over cores
    m_bt = counts.max(axis=0)                      # [BPC, NSUB]
    len_t = m_bt.sum(axis=0)                       # slots per stream
    ncol_t = ((len_t + 127) // 128).astype(np.int64)
    col_off_t = np.concatenate([[0], np.cumsum(ncol_t)[:-1]])
    NCOL = int(ncol_t.sum())
    slot_off = np.zeros((BPC, NSUB), np.int64)
    for t in range(NSUB):
        slot_off[:, t] = np.concatenate([[0], np.cumsum(m_bt[:, t])[:-1]])

    # occurrences: (t, c, b, lo, hi), c = stream-local column,
    # [lo, hi) = column-local slot range; ordered by (t, c, b)
    occs = []
    occ_of_block = [[] for _ in range(BPC)]
    for t in range(NSUB):
        for b in range(BPC):
            s0 = int(slot_off[b, t])
            s1 = s0 + int(m_bt[b, t])
            if s1 == s0:
                continue
            for c in range(s0 // 128, (s1 - 1) // 128 + 1):
                lo = max(s0, c * 128) - c * 128
                hi = min(s1, (c + 1) * 128) - c * 128
                occs.append((t, c, b, lo, hi))
    occs.sort(key=lambda o: (o[0], o[1], o[2]))
    for j, (t, c, b, lo, hi) in enumerate(occs):
        occ_of_block[b].append(j)
    NOCC = len(occs)

    # per-core arrays: gather idx per global slot; dstl per occurrence
    NI = NCOL * 128
    idx_all = np.zeros((NCORES, NI), np.int64)
    dstl_occ = np.full((NCORES, NOCC, P), -1.0, np.float32)
    for k in range(NCORES):
        for t in range(NSUB):
            base_i = int(col_off_t[t]) * 128
            for b in range(BPC):
                n = int(counts[k, b, t])
                if n == 0:
                    continue
                s0 = int(starts[k, b, t])
                g0 = base_i + int(slot_off[b, t])
                idx_all[k, g0 : g0 + n] = loc_s[s0 : s0 + n]
        for j, (t, c, b, lo, hi) in enumerate(occs):
            sg0 = c * 128 + lo - int(slot_off[b, t])  # offset within group
            n = int(counts[k, b, t])
            m = min(max(n - sg0, 0), hi - lo)
            if m > 0:
                s0 = int(starts[k, b, t])
                dstl_occ[k, j, lo : lo + m] = dstl_s[s0 + sg0 : s0 + sg0 + m]
    idx16 = np.stack([_wrap_idx16(idx_all[k]) for k in range(NCORES)])
    dstl_t = np.ascontiguousarray(dstl_occ.transpose(0, 2, 1)).astype(bf16)

    # windows: CW columns per (stream t, w); occurrences grouped per window
    win_range = {}
    occ_win = []
    win_occ_range = {}
    for t in range(NSUB):
        for w in range(int((ncol_t[t] + CW - 1) // CW)):
            c0 = int(w * CW)
            c1 = int(min((w + 1) * CW, ncol_t[t]))
            win_range[(t, w)] = (c0, c1)
    for j, (t, c, b, lo, hi) in enumerate(occs):
        w = c // CW
        occ_win.append((t, w))
        if (t, w) not in win_occ_range:
            win_occ_range[(t, w)] = [j, j + 1]
        else:
            win_occ_range[(t, w)][1] = j + 1

    # extended-edge (leader) layout: uniform width WU per dst
    deg = np.bincount(col, minlength=NPAD) + 1
    WU = int(deg.max())
    elog_src = np.full((NCORES, P, BPC, WU), -1, np.int64)
    dorder = np.argsort(col, kind="stable")
    row_d = row[dorder]
    dst_starts = np.concatenate([[0], np.cumsum(np.bincount(col, minlength=NPAD))])
    for k in range(NCORES):
        for b in range(BPC):
            for p in range(P):
                d = int(inv[k, b, p])
                if d < 0:
                    continue
                s0, s1 = int(dst_starts[d]), int(dst_starts[d + 1])
                m = s1 - s0
                elog_src[k, p, b, 0] = d
                if m > 0:
                    elog_src[k, p, b, 1 : 1 + m] = row_d[s0:s1]
    elog_src = elog_src.reshape(NCORES, P, BPC * WU)
    srcp1 = np.where(elog_src >= 0, elog_src + 1, 0).astype(np.float32)

    return dict(
        occs=occs, occ_of_block=occ_of_block, NOCC=NOCC, NCOL=NCOL,
        col_off_t=col_off_t, ncol_t=ncol_t, win_range=win_range,
        occ_win=occ_win, win_occ_range=win_occ_range,
        idx16=idx16, dstl_t=dstl_t,
        WU=WU, elog_src=elog_src, srcp1=srcp1,
        node2kbp=node2kbp, inv=inv,
    )


# ---------------------------------------------------------------------------
# launch 1: logits + segment mean + MLP -> reports
# ---------------------------------------------------------------------------

def _build_l1(pp):
    occs = pp["occs"]
    NOCC = pp["NOCC"]
    NCOL = pp["NCOL"]
    NI16 = NCOL * 8
    OH_W = max(j1 - j0 for (j0, j1) in pp["win_occ_range"].values())

    nc = bacc.Bacc("TRN2", target_bir_lowering=False, debug=False,
                   num_devices=NCORES)
    xtab_d = nc.dram_tensor("xtab", [NPAD, 2 * H], dt.bfloat16, kind="ExternalInput")
    xf_d = nc.dram_tensor("xf", [NPC, H], dt.float32, kind="ExternalInput")
    idx_d = nc.dram_tensor("idx16", [P, NI16], dt.int16, kind="ExternalInput")
    dstl_d = nc.dram_tensor("dstl", [P, NOCC], dt.bfloat16, kind="ExternalInput")
    wrep_d = nc.dram_tensor("wrep", [P, H], dt.float32, kind="ExternalInput")
    blead_d = nc.dram_tensor("blead", [P, 1], dt.float32, kind="ExternalInput")
    w1_d = nc.dram_tensor("w1", [H, H], dt.bfloat16, kind="ExternalInput")
    b1_d = nc.dram_tensor("b1", [P, 1], dt.float32, kind="ExternalInput")
    w2_d = nc.dram_tensor("w2", [H, C], dt.bfloat16, kind="ExternalInput")
    b2_d = nc.dram_tensor("b2", [1, C], dt.bfloat16, kind="ExternalInput")

    logits_o = nc.dram_tensor("logits_o", [P, BPC], dt.float32,
                              kind="ExternalOutput")
    reports_o = nc.dram_tensor("reports_o", [P, NPC], dt.float32,
                               kind="ExternalOutput")

    STAGE_B = 14  # blocks per reports staging buffer (98 = 7*14)

    with tile.TileContext(nc) as tc:
        with (
            tc.tile_pool(name="const", bufs=1) as cp,
            tc.tile_pool(name="xt", bufs=3) as xp,
            tc.tile_pool(name="g", bufs=6) as gp,
            tc.tile_pool(name="oh", bufs=4) as op_,
            tc.tile_pool(name="small", bufs=4) as sp,
            tc.tile_pool(name="stage", bufs=2) as stp,
            tc.tile_pool(name="sums_ps", bufs=3, space="PSUM") as sums_pp,
            tc.tile_pool(name="mlp_ps", bufs=4, space="PSUM") as mlp_pp,
        ):
            idx16_t = cp.tile([P, NI16], dt.int16)
            nc.sync.dma_start(idx16_t[:], idx_d[:, :])
            dstl_t = cp.tile([P, NOCC], dt.bfloat16)
            nc.sync.dma_start(dstl_t[:], dstl_d[:, :])
            wrep_t = cp.tile([P, H], dt.float32)
            nc.sync.dma_start(wrep_t[:], wrep_d[:, :])
            blead_t = cp.tile([P, 1], dt.float32)
            nc.sync.dma_start(blead_t[:], blead_d[:, :])
            w1_t = cp.tile([H, H], dt.bfloat16)
            nc.sync.dma_start(w1_t[:], w1_d[:, :])
            b1_t = cp.tile([P, 1], dt.float32)
            nc.sync.dma_start(b1_t[:], b1_d[:, :])
            w2_t = cp.tile([H, C], dt.bfloat16)
            nc.sync.dma_start(w2_t[:], w2_d[:, :])
            b2_t = cp.tile([1, C], dt.bfloat16)
            nc.sync.dma_start(b2_t[:], b2_d[:, :])

            iota_i = cp.tile([P, P], dt.int32)
            nc.gpsimd.iota(iota_i[:], pattern=[[1, P]], base=0,
                           channel_multiplier=0)
            iota_f = cp.tile([P, P], dt.bfloat16)
            nc.vector.tensor_copy(iota_f[:], iota_i[:])
            ident_f = cp.tile([P, P], dt.bfloat16)
            make_identity(nc, ident_f[:])
            ones_row = cp.tile([1, P], dt.bfloat16)
            nc.vector.memset(ones_row[:], 1.0)

            logits_sb = cp.tile([P, BPC], dt.float32)

            # ---- logits ----------------------------------------------------
            for b in range(BPC):
                xt = xp.tile([P, H], dt.float32)
                nc.sync.dma_start(xt[:], xf_d[b * P : (b + 1) * P, :])
                tmp = xp.tile([P, H], dt.float32, tag="ltmp")
                nc.vector.tensor_tensor(out=tmp[:], in0=xt[:], in1=wrep_t[:],
                                        op=mybir.AluOpType.mult)
                nc.vector.reduce_sum(out=logits_sb[:, b : b + 1], in_=tmp[:],
                                     axis=mybir.AxisListType.X)
            logits_out = sp.tile([P, BPC], dt.float32, tag="lgout")
            nc.vector.tensor_scalar_add(logits_out[:], logits_sb[:],
                                        blead_t[:, :1])
            nc.sync.dma_start(logits_o[:, :], logits_out[:])

            # ---- gather windows + one_hot ----------------------------------
            win_tiles = {}
            win_range = pp["win_range"]
            win_occ_range = pp["win_occ_range"]
            col_off_t = pp["col_off_t"]

            def ensure_window(t, w):
                if (t, w) in win_tiles:
                    return win_tiles[(t, w)]
                c0, c1 = win_range[(t, w)]
                ncw = c1 - c0
                gc0 = int(col_off_t[t]) + c0      # global column
                G = gp.tile([P, CW, 2 * H], dt.bfloat16, tag="g")
                nc.gpsimd.dma_gather(
                    out_ap=G[:, :ncw, :],
                    in_ap=xtab_d[t * SUB : (t + 1) * SUB, :],
                    idxs_ap=idx16_t[:, gc0 * 8 : (gc0 + ncw) * 8],
                    num_idxs=ncw * 128,
                    num_idxs_reg=ncw * 128,
                    elem_size=2 * H,
                    single_packet=False,
                )
                j0, j1 = win_occ_range[(t, w)]
                no = j1 - j0
                oh = op_.tile([P, OH_W, P], dt.bfloat16, tag="oh")
                sl = dstl_t[:, j0:j1]
                iota_b = bass.AP(iota_f.tensor, 0,
                                 [iota_f[:].ap[0], [0, no], [1, P]])
                dstl_b = bass.AP(sl.tensor, sl.offset,
                                 [sl.ap[0], [1, no], [0, P]])
                nc.vector.tensor_tensor(out=oh[:, :no, :], in0=iota_b,
                                        in1=dstl_b,
                                        op=mybir.AluOpType.is_equal)
                win_tiles[(t, w)] = (G, oh, c0, j0)
                return win_tiles[(t, w)]

            stage_t = None
            nblk = 0
            for b in range(BPC):
                ol = pp["occ_of_block"][b]
                sums_ps = sums_pp.tile([P, H + 1], dt.float32, space="PSUM",
                                       tag="sums")
                nref = len(ol)
                for i, j in enumerate(ol):
                    t, c, _b, lo, hi = occs[j]
                    w = c // CW
                    G, oh, c0, j0 = ensure_window(t, w)
                    nc.tensor.matmul(out=sums_ps[:], lhsT=oh[:, j - j0, :],
                                     rhs=G[:, c - c0, 0 : H + 1],
                                     start=(i == 0),
                                     stop=(i == nref - 1))

                c1t = sp.tile([P, 1], dt.float32, tag="c1")
                nc.vector.tensor_scalar_max(c1t[:], sums_ps[:, H : H + 1], 1.0)
                rc = sp.tile([P, 1], dt.float32, tag="rc")
                nc.vector.reciprocal(rc[:], c1t[:])
                mean_sb = sp.tile([P, P], dt.bfloat16, tag="mean")
                nc.vector.tensor_scalar_mul(mean_sb[:], sums_ps[:, 0:H],
                                            rc[:, :1])

                meanT_ps = mlp_pp.tile([P, P], dt.bfloat16, space="PSUM",
                                       tag="mlp")
                nc.tensor.transpose(meanT_ps[:], mean_sb[:], ident_f[:])
                meanT_sb = sp.tile([P, P], dt.bfloat16, tag="meanT")
                nc.scalar.activation(meanT_sb[:], meanT_ps[:],
                                     mybir.ActivationFunctionType.Copy)
                hpre_ps = mlp_pp.tile([P, P], dt.float32, space="PSUM",
                                      tag="mlp")
                nc.tensor.matmul(out=hpre_ps[:], lhsT=w1_t[:], rhs=meanT_sb[:],
                                 start=True, stop=True)
                hT_sb = sp.tile([P, P], dt.bfloat16, tag="hT")
                nc.scalar.activation(hT_sb[:], hpre_ps[:],
                                     mybir.ActivationFunctionType.Gelu,
                                     bias=b1_t[:, :1])
                rep_ps = mlp_pp.tile([P, P], dt.float32, space="PSUM",
                                     tag="mlp")
                nc.tensor.matmul(out=rep_ps[:], lhsT=hT_sb[:], rhs=w2_t[:],
                                 start=True, stop=False)
                nc.tensor.matmul(out=rep_ps[:], lhsT=ones_row[:], rhs=b2_t[:],
                                 start=False, stop=True)

                sj = b % STAGE_B
                if sj == 0:
                    nblk = min(STAGE_B, BPC - b)
                    stage_t = stp.tile([P, STAGE_B * P], dt.float32,
                                       tag="stage")
                nc.scalar.activation(stage_t[:, sj * P : (sj + 1) * P],
                                     rep_ps[:],
                                     mybir.ActivationFunctionType.Copy)
                if sj == nblk - 1:
                    b0 = b - sj
                    nc.sync.dma_start(
                        reports_o[:, b0 * P : (b0 + sj + 1) * P],
                        stage_t[:, : (sj + 1) * P],
                    )
    nc.compile()
    return nc


# ---------------------------------------------------------------------------
# launch 2: leader election + output gather
# ---------------------------------------------------------------------------

def _build_l23(pp):
    WU = pp["WU"]
    SW = BPC * WU
    nc = bacc.Bacc("TRN2", target_bir_lowering=False, debug=False,
                   num_devices=NCORES)
    ep_d = nc.dram_tensor("epad", [P, SW], dt.float32, kind="ExternalInput")
    sp1_d = nc.dram_tensor("srcp1", [P, SW], dt.float32, kind="ExternalInput")
    rep_d = nc.dram_tensor("repfull", [NPAD, C], dt.float32,
                           kind="ExternalInput")
    out_o = nc.dram_tensor("gath_o", [P, BPC * C], dt.float32,
                           kind="ExternalOutput")

    STAGE_B = 14

    with tile.TileContext(nc) as tc:
        with (
            tc.tile_pool(name="sb", bufs=1) as sb,
            tc.tile_pool(name="rows", bufs=4) as rp_,
            tc.tile_pool(name="stage", bufs=2) as stp,
        ):
            ep = sb.tile([P, BPC, WU], dt.float32)
            nc.sync.dma_start(ep[:], ep_d[:, :].rearrange("p (b w) -> p b w", w=WU))
            sp1 = sb.tile([P, BPC, WU], dt.float32)
            nc.sync.dma_start(sp1[:], sp1_d[:, :].rearrange("p (b w) -> p b w", w=WU))

            sm = sb.tile([P, BPC], dt.float32)
            nc.vector.reduce_max(out=sm[:], in_=ep[:], axis=mybir.AxisListType.X)
            mask = sb.tile([P, BPC, WU], dt.float32)
            sm_b = bass.AP(sm.tensor, 0, [sm[:].ap[0], [1, BPC], [0, WU]])
            nc.vector.tensor_tensor(out=mask[:], in0=ep[:], in1=sm_b,
                                    op=mybir.AluOpType.is_equal)
            cand = sb.tile([P, BPC, WU], dt.float32)
            nc.vector.tensor_tensor(out=cand[:], in0=mask[:], in1=sp1[:],
                                    op=mybir.AluOpType.mult)
            lp1 = sb.tile([P, BPC], dt.float32)
            nc.vector.reduce_max(out=lp1[:], in_=cand[:],
                                 axis=mybir.AxisListType.X)
            leadf = sb.tile([P, BPC], dt.float32)
            nc.vector.tensor_scalar(
                out=leadf[:], in0=lp1[:], scalar1=-1.0, scalar2=0.0,
                op0=mybir.AluOpType.add, op1=mybir.AluOpType.max,
            )
            leadi = sb.tile([P, BPC], dt.int32)
            nc.vector.tensor_copy(leadi[:], leadf[:])

            stage_t = None
            nblk = 0
            for b in range(BPC):
                rows = rp_.tile([P, C], dt.float32, tag="rows")
                nc.gpsimd.indirect_dma_start(
                    out=rows[:],
                    out_offset=None,
                    in_=rep_d[:, :],
                    in_offset=bass.IndirectOffsetOnAxis(
                        ap=leadi[:, b : b + 1], axis=0),
                )
                sj = b % STAGE_B
                if sj == 0:
                    nblk = min(STAGE_B, BPC - b)
                    stage_t = stp.tile([P, STAGE_B * C], dt.float32,
                                       tag="stage")
                nc.vector.tensor_copy(stage_t[:, sj * C : (sj + 1) * C],
                                      rows[:])
                if sj == nblk - 1:
                    b0 = b - sj
                    nc.sync.dma_start(
                        out_o[:, b0 * C : (b0 + sj + 1) * C],
                        stage_t[:, : (sj + 1) * C],
                    )
    nc.compile()
    return nc


# ---------------------------------------------------------------------------

_CACHE = {}


def _get(key, fn):
    if key not in _CACHE:
        _CACHE[key] = fn()
    return _CACHE[key]


def kernel(x, edge_index, w_lead, b_lead, w1, b1, w2, b2):
    x = np.asarray(x, np.float32)
    N = x.shape[0]
    assert N == 100000 and x.shape[1] == H

    ekey = hashlib.md5(np.asarray(edge_index).tobytes()).hexdigest()
    pp = _get(("pp", ekey), lambda: _preprocess(edge_index))

    xpad = np.zeros((NPAD, H), np.float32)
    xpad[:N] = x
    inv = pp["inv"]                       # [NCORES, BPC, P] orig node or -1
    inv_flat = inv.reshape(NCORES, NPC)
    xtab = np.zeros((NPAD, 2 * H), bf16)
    xtab[:, 0:H] = xpad.astype(bf16)
    xtab[:, H] = bf16(1.0)
    wrep = np.tile(np.asarray(w_lead, np.float32)[None, :], (P, 1))
    blead = np.full((P, 1), np.float32(b_lead), np.float32)
    w1f = np.ascontiguousarray(np.asarray(w1, np.float32).astype(bf16))
    b1c = np.ascontiguousarray(np.asarray(b1, np.float32).reshape(H, 1))
    w2f = np.ascontiguousarray(np.asarray(w2, np.float32).astype(bf16))
    b2r = np.ascontiguousarray(
        np.asarray(b2, np.float32).reshape(1, C).astype(bf16))

    # ---- launch 1 ----------------------------------------------------------
    nc1 = _get(("l1", ekey), lambda: _build_l1(pp))
    in_maps = []
    for k in range(NCORES):
        in_maps.append({
            "xtab": xtab,
            "xf": np.ascontiguousarray(
                np.where((inv_flat[k] >= 0)[:, None],
                         xpad[np.maximum(inv_flat[k], 0)], 0.0)),
            "idx16": pp["idx16"][k],
            "dstl": pp["dstl_t"][k],
            "wrep": wrep,
            "blead": blead,
            "w1": w1f,
            "b1": b1c,
            "w2": w2f,
            "b2": b2r,
        })
    r1 = run_bass_kernel_spmd(nc1, in_maps, core_ids=CORES)

    logits_full = np.zeros(NPAD, np.float32)
    reports_full = np.zeros((NPAD, C), np.float32)
    for k in range(NCORES):
        lg = r1.results[k]["logits_o"]            # [P, BPC]
        rp = r1.results[k]["reports_o"]           # [P, NPC] block-major cols
        m = inv_flat[k] >= 0
        ids = inv_flat[k][m]
        logits_full[ids] = lg.T.reshape(-1)[m]
        reports_full[ids] = rp.reshape(P, BPC, C).transpose(1, 0, 2).reshape(
            NPC, C)[m]

    # ---- launch 2+3: leader election + gather -------------------------------
    nc23 = _get(("l23", ekey), lambda: _build_l23(pp))
    es = pp["elog_src"]
    in_maps2 = []
    for k in range(NCORES):
        ep = np.where(es[k] >= 0, logits_full[np.maximum(es[k], 0)], NEG)
        in_maps2.append({
            "epad": np.ascontiguousarray(ep.astype(np.float32)),
            "srcp1": pp["srcp1"][k],
            "repfull": reports_full,
        })
    r2 = run_bass_kernel_spmd(nc23, in_maps2, core_ids=CORES)

    out = np.zeros((N, C), np.float32)
    for k in range(NCORES):
        g = r2.results[k]["gath_o"].reshape(P, BPC, C)
        node_rows = g.transpose(1, 0, 2).reshape(NPC, C)
        m = inv_flat[k] >= 0
        out[inv_flat[k][m]] = node_rows[m]
    return out



# revision 5
# speedup vs baseline: 2.4377x; 2.4377x over previous
"""Trainium2 Bass kernel for nn_DecentralizedCoordinator (GNN message passing).

Strategy (8 NeuronCores, SPMD), v2:
- Nodes sharded by id: core k owns 98 blocks of 128 dst nodes (12544/core).
- Edges partitioned by destination (core, block); per-block slot streams are
  column-aligned (multiples of 128). Host materializes the halo / edge-source
  feature table ET[k] = x[src] in slot order (bf16, partition-major layout)
  — the "all-gather + edge partition" sharding step — so the device streams
  it SEQUENTIALLY at line rate instead of descriptor-bound random gathers.
- Device L1 (all value arithmetic):
  logits = x·w_lead + b_lead per owned node (vector),
  segment_sum via one-hot matmul per (block, column) occurrence into PSUM,
  mean = sums * recip (recip = 1/max(indeg,1), host-known structure),
  MLP: meanT -> w1 -> gelu(+b1) -> w2 (+b2) -> reports, staged DMA out.
- Host between launches: assemble per-dst padded logits layout (epad/srcp1)
  from L1 logits — pure indexed reshuffle, pattern known at preprocess.
- Device L2: leader election (reduce_max / is_equal / mult / reduce_max,
  exact reference tie-break), then the value-dependent gather
  reports[leader] via indirect DMA; output written partition-major.

Host only shards/permutes/reshuffles by precomputed indices; every operation
on runtime values (sums, means, MLP, comparisons, final gather) is on device.
"""
import hashlib
import sys

import numpy as np
import ml_dtypes

sys.path.insert(0, "/opt/trn_rl_repo")

import concourse.bass as bass
import concourse.tile as tile
from concourse import bacc, mybir
from concourse.bass_utils import run_bass_kernel_spmd
from concourse.masks import make_identity

dt = mybir.dt
bf16 = ml_dtypes.bfloat16

P = 128
NCORES = 8
BPC = 98                 # dst blocks per core
NPC = BPC * P            # 12544 nodes per core
NPAD = NCORES * NPC      # 100352 padded node count
N_NODES = 100000
H = 128
C = 128
CW = 64                  # ET window width (columns of 128 slots)
SB = 7                   # blocks per MLP/output stage (98 = 14*7)
NEG = -3.0e38

CORES = list(range(NCORES))


def _assign_nodes(col, n_nodes):
    """Balanced node -> (core, block, slot): equalize per-block in-degree sums
    across cores. Returns node2kbp [N,3] and inv [NCORES,BPC,P]."""
    indeg = np.bincount(col, minlength=n_nodes)
    order = np.argsort(-indeg, kind="stable")
    node2kbp = np.zeros((n_nodes, 3), np.int64)
    inv = np.full((NCORES, BPC, P), -1, np.int64)
    BIG = 1 << 40
    for b in range(BPC):
        sl = order[b * NCORES * P: (b + 1) * NCORES * P]
        loads = np.zeros(NCORES, np.int64)
        caps = np.full(NCORES, P, np.int64)
        slots = np.zeros(NCORES, np.int64)
        for nd in sl:
            cost = loads.astype(np.float64).copy()
            cost[caps == 0] = BIG
            kbest = int(np.argmin(cost))
            loads[kbest] += indeg[nd]
            caps[kbest] -= 1
            p = int(slots[kbest])
            slots[kbest] += 1
            node2kbp[nd] = (kbest, b, p)
            inv[kbest, b, p] = nd
    return node2kbp, inv


def _preprocess(edge_index):
    row = np.asarray(edge_index[0], np.int64)
    col = np.asarray(edge_index[1], np.int64)

    node2kbp, inv = _assign_nodes(col, N_NODES)
    core = node2kbp[col, 0]
    blk = node2kbp[col, 1]
    dstl = node2kbp[col, 2]

    gkey = core * BPC + blk
    order = np.argsort(gkey, kind="stable")
    src_s = row[order]
    dstl_s = dstl[order]
    counts = np.bincount(gkey, minlength=NCORES * BPC).reshape(NCORES, BPC)
    starts = np.concatenate(
        [[0], np.cumsum(counts.reshape(-1))[:-1]]).reshape(counts.shape)

    m_b = counts.max(axis=0)                       # [BPC]
    mcols = (m_b + P - 1) // P                     # columns per block
    colbase = np.concatenate([[0], np.cumsum(mcols)[:-1]])
    NCOL = int(mcols.sum())

    # occurrences: one per (block, column), ordered by column
    occbase = colbase.copy()                       # occ j == global column
    NOCC = NCOL
    occ_block = np.zeros(NOCC, np.int64)
    for b in range(BPC):
        occ_block[colbase[b]: colbase[b] + mcols[b]] = b

    # per-core slot source ids + per-occurrence dst-slot rows
    srcidx = np.full((NCORES, NCOL * P), N_NODES, np.int64)
    dstl_occ = np.full((NCORES, P, NOCC), -1.0, np.float32)
    for k in range(NCORES):
        for b in range(BPC):
            n = int(counts[k, b])
            if n == 0:
                continue
            s0 = int(starts[k, b])
            g0 = int(colbase[b]) * P
            srcidx[k, g0: g0 + n] = src_s[s0: s0 + n]
            pos = np.arange(n)
            cc = int(colbase[b]) + pos // P
            pp = pos % P
            dstl_occ[k, pp, cc] = dstl_s[s0: s0 + n]
    dstl_bf = dstl_occ.astype(bf16)

    # recip of true in-degree per owned node, [NCORES, P, BPC]
    indeg = np.bincount(col, minlength=N_NODES).astype(np.float32)
    cnt = np.where(inv >= 0, indeg[np.maximum(inv, 0)], 0.0)  # [NC,BPC,P]
    recip = (1.0 / np.maximum(cnt, 1.0)).transpose(0, 2, 1).astype(
        np.float32)                                # [NC, P, BPC]
    recip = np.ascontiguousarray(recip)

    # leader-election padded layout (extended edges with self loops)
    deg = np.bincount(col, minlength=NPAD) + 1
    WU = int(deg.max())
    elog_src = np.full((NCORES, P, BPC, WU), -1, np.int64)
    dorder = np.argsort(col, kind="stable")
    row_d = row[dorder]
    dst_starts = np.concatenate(
        [[0], np.cumsum(np.bincount(col, minlength=NPAD))])
    for k in range(NCORES):
        for b in range(BPC):
            for p in range(P):
                d = int(inv[k, b, p])
                if d < 0:
                    continue
                s0, s1 = int(dst_starts[d]), int(dst_starts[d + 1])
                m = s1 - s0
                elog_src[k, p, b, 0] = d
                if m > 0:
                    elog_src[k, p, b, 1: 1 + m] = row_d[s0:s1]
    elog_src = elog_src.reshape(NCORES, P, BPC * WU)
    srcp1 = np.where(elog_src >= 0, elog_src + 1, 0).astype(np.float32)

    return dict(
        NCOL=NCOL, NOCC=NOCC, mcols=mcols, colbase=colbase,
        occ_block=occ_block, srcidx=srcidx, dstl_bf=dstl_bf, recip=recip,
        WU=WU, elog_src=elog_src, srcp1=srcp1,
        node2kbp=node2kbp, inv=inv,
    )


# ---------------------------------------------------------------------------
# launch 1: logits + segment mean + MLP -> reports
# ---------------------------------------------------------------------------

def _build_l1(pp):
    NCOL = pp["NCOL"]
    NOCC = pp["NOCC"]
    mcols = pp["mcols"]
    colbase = pp["colbase"]
    NW = (NCOL + CW - 1) // CW
    # occurrences per window (occ j == column j)
    OH_W = max(min((w + 1) * CW, NCOL) - w * CW for w in range(NW))

    nc = bacc.Bacc("TRN2", target_bir_lowering=False, debug=False,
                   num_devices=NCORES)
    et_d = nc.dram_tensor("et", [P, NCOL * H], dt.bfloat16,
                          kind="ExternalInput")
    xf_d = nc.dram_tensor("xf2", [P, BPC * H], dt.float32,
                          kind="ExternalInput")
    dstl_d = nc.dram_tensor("dstl", [P, NOCC], dt.bfloat16,
                            kind="ExternalInput")
    recip_d = nc.dram_tensor("recip", [P, BPC], dt.float32,
                             kind="ExternalInput")
    wrep_d = nc.dram_tensor("wrep", [P, H], dt.float32, kind="ExternalInput")
    blead_d = nc.dram_tensor("blead", [P, 1], dt.float32,
                             kind="ExternalInput")
    w1_d = nc.dram_tensor("w1", [H, H], dt.bfloat16, kind="ExternalInput")
    b1_d = nc.dram_tensor("b1", [P, 1], dt.float32, kind="ExternalInput")
    w2_d = nc.dram_tensor("w2", [H, C], dt.bfloat16, kind="ExternalInput")
    b2_d = nc.dram_tensor("b2", [1, C], dt.bfloat16, kind="ExternalInput")

    logits_o = nc.dram_tensor("logits_o", [P, BPC], dt.float32,
                              kind="ExternalOutput")
    reports_o = nc.dram_tensor("reports_o", [P, BPC * C], dt.float32,
                               kind="ExternalOutput")

    with tile.TileContext(nc) as tc:
        with (
            tc.tile_pool(name="const", bufs=1) as cp,
            tc.tile_pool(name="xf", bufs=2) as xp,
            tc.tile_pool(name="g", bufs=3) as gp,
            tc.tile_pool(name="oh", bufs=3) as op_,
            tc.tile_pool(name="small", bufs=3) as sp,
            tc.tile_pool(name="mstage", bufs=2) as mp,
            tc.tile_pool(name="stage", bufs=2) as stp,
            tc.tile_pool(name="sums_ps", bufs=2, space="PSUM") as sums_pp,
            tc.tile_pool(name="tr_ps", bufs=2, space="PSUM") as tr_pp,
            tc.tile_pool(name="hpre_ps", bufs=1, space="PSUM") as hpre_pp,
            tc.tile_pool(name="rep_ps", bufs=2, space="PSUM") as rep_pp,
        ):
            dstl_t = cp.tile([P, NOCC], dt.bfloat16)
            nc.sync.dma_start(dstl_t[:], dstl_d[:, :])
            recip_t = cp.tile([P, BPC], dt.float32)
            nc.sync.dma_start(recip_t[:], recip_d[:, :])
            wrep_t = cp.tile([P, H], dt.float32)
            nc.sync.dma_start(wrep_t[:], wrep_d[:, :])
            blead_t = cp.tile([P, 1], dt.float32)
            nc.sync.dma_start(blead_t[:], blead_d[:, :])
            w1_t = cp.tile([H, H], dt.bfloat16)
            nc.sync.dma_start(w1_t[:], w1_d[:, :])
            b1_t = cp.tile([P, 1], dt.float32)
            nc.sync.dma_start(b1_t[:], b1_d[:, :])
            w2_t = cp.tile([H, C], dt.bfloat16)
            nc.sync.dma_start(w2_t[:], w2_d[:, :])
            b2_t = cp.tile([1, C], dt.bfloat16)
            nc.sync.dma_start(b2_t[:], b2_d[:, :])

            iota_i = cp.tile([P, P], dt.int32)
            nc.gpsimd.iota(iota_i[:], pattern=[[1, P]], base=0,
                           channel_multiplier=0)
            iota_f = cp.tile([P, P], dt.bfloat16)
            nc.vector.tensor_copy(iota_f[:], iota_i[:])
            ident_f = cp.tile([P, P], dt.bfloat16)
            make_identity(nc, ident_f[:])
            ones_row = cp.tile([1, P], dt.bfloat16)
            nc.vector.memset(ones_row[:], 1.0)

            # ---- logits (xf2 partition-major: [p, b*H+f]) ------------------
            logits_sb = cp.tile([P, BPC], dt.float32)
            XC = 14
            for ch in range((BPC + XC - 1) // XC):
                b0 = ch * XC
                nb = min(XC, BPC - b0)
                xfc = xp.tile([P, XC * H], dt.float32, tag="xfc")
                nc.sync.dma_start(xfc[:, : nb * H],
                                  xf_d[:, b0 * H: (b0 + nb) * H])
                for j in range(nb):
                    tmp = xp.tile([P, H], dt.float32, tag="ltmp")
                    nc.vector.tensor_tensor(
                        out=tmp[:], in0=xfc[:, j * H: (j + 1) * H],
                        in1=wrep_t[:], op=mybir.AluOpType.mult)
                    nc.vector.reduce_sum(
                        out=logits_sb[:, b0 + j: b0 + j + 1], in_=tmp[:],
                        axis=mybir.AxisListType.X)
            logits_out = sp.tile([P, BPC], dt.float32, tag="lgout")
            nc.vector.tensor_scalar_add(logits_out[:], logits_sb[:],
                                        blead_t[:, :1])
            nc.sync.dma_start(logits_o[:, :], logits_out[:])

            # ---- ET windows + one-hots ------------------------------------
            win_tiles = {}

            def ensure_window(w):
                if w in win_tiles:
                    return win_tiles[w]
                c0 = w * CW
                c1 = min((w + 1) * CW, NCOL)
                ncw = c1 - c0
                G = gp.tile([P, CW * H], dt.bfloat16, tag="g")
                nc.sync.dma_start(G[:, : ncw * H],
                                  et_d[:, c0 * H: c1 * H])
                oh = op_.tile([P, OH_W, P], dt.bfloat16, tag="oh")
                sl = dstl_t[:, c0:c1]
                iota_b = bass.AP(iota_f.tensor, 0,
                                 [iota_f[:].ap[0], [0, ncw], [1, P]])
                dstl_b = bass.AP(sl.tensor, sl.offset,
                                 [sl.ap[0], [1, ncw], [0, P]])
                nc.vector.tensor_tensor(out=oh[:, :ncw, :], in0=iota_b,
                                        in1=dstl_b,
                                        op=mybir.AluOpType.is_equal)
                win_tiles[w] = (G, oh, c0)
                return win_tiles[w]

            MB = 4                     # blocks per MLP chunk (1 PSUM bank)
            OB = 28                    # blocks per output-staging DMA (MB | OB)
            meanT_stage = None
            stage_out = None
            for b in range(BPC):
                mj = b % MB
                nmb = min(MB, BPC - (b - mj))
                sj = b % OB
                if mj == 0:
                    meanT_stage = mp.tile([P, MB * P], dt.bfloat16,
                                          tag="meanT")
                if sj == 0:
                    stage_out = stp.tile([P, OB * C], dt.float32, tag="st")

                sums_ps = sums_pp.tile([P, H], dt.float32, space="PSUM",
                                       tag="sums")
                nmm = int(mcols[b])
                for i in range(nmm):
                    c = int(colbase[b]) + i
                    G, oh, c0 = ensure_window(c // CW)
                    nc.tensor.matmul(out=sums_ps[:],
                                     lhsT=oh[:, c - c0, :],
                                     rhs=G[:, (c - c0) * H: (c - c0 + 1) * H],
                                     start=(i == 0),
                                     stop=(i == nmm - 1))

                mean_sb = sp.tile([P, P], dt.bfloat16, tag="mean")
                nc.vector.tensor_scalar_mul(mean_sb[:], sums_ps[:],
                                            recip_t[:, b: b + 1])
                meanT_ps = tr_pp.tile([P, P], dt.bfloat16, space="PSUM",
                                      tag="tr")
                nc.tensor.transpose(meanT_ps[:], mean_sb[:], ident_f[:])
                nc.scalar.activation(meanT_stage[:, mj * P: (mj + 1) * P],
                                     meanT_ps[:],
                                     mybir.ActivationFunctionType.Copy)

                if mj == nmb - 1:
                    bm = b - mj
                    hpre_ps = hpre_pp.tile([P, MB * H], dt.float32,
                                           space="PSUM", tag="hpre")
                    nc.tensor.matmul(out=hpre_ps[:, : nmb * H], lhsT=w1_t[:],
                                     rhs=meanT_stage[:, : nmb * P],
                                     start=True, stop=True)
                    hT_stage = mp.tile([P, MB * H], dt.bfloat16, tag="hT")
                    nc.scalar.activation(hT_stage[:, : nmb * H],
                                         hpre_ps[:, : nmb * H],
                                         mybir.ActivationFunctionType.Gelu,
                                         bias=b1_t[:, :1])
                    for j in range(nmb):
                        rep_ps = rep_pp.tile([P, C], dt.float32,
                                             space="PSUM", tag="rep")
                        nc.tensor.matmul(out=rep_ps[:],
                                         lhsT=hT_stage[:, j * H: (j + 1) * H],
                                         rhs=w2_t[:], start=True, stop=False)
                        nc.tensor.matmul(out=rep_ps[:], lhsT=ones_row[:],
                                         rhs=b2_t[:], start=False, stop=True)
                        nc.scalar.activation(
                            stage_out[:, ((bm + j) % OB) * C:
                                      ((bm + j) % OB + 1) * C], rep_ps[:],
                            mybir.ActivationFunctionType.Copy)
                if sj == OB - 1 or b == BPC - 1:
                    b0 = b - sj
                    nc.sync.dma_start(
                        reports_o[:, b0 * C: (b + 1) * C],
                        stage_out[:, : (sj + 1) * C])
    nc.compile()
    return nc


# ---------------------------------------------------------------------------
# launch 2: leader election + output gather
# ---------------------------------------------------------------------------

def _build_l2(pp, batched_indirect):
    WU = pp["WU"]
    SW = BPC * WU
    nc = bacc.Bacc("TRN2", target_bir_lowering=False, debug=False,
                   num_devices=NCORES)
    ep_d = nc.dram_tensor("epad", [P, SW], dt.float32, kind="ExternalInput")
    sp1_d = nc.dram_tensor("srcp1", [P, SW], dt.float32,
                           kind="ExternalInput")
    rep_d = nc.dram_tensor("repfull", [NPAD, C], dt.float32,
                           kind="ExternalInput")
    out_o = nc.dram_tensor("gath_o", [P, BPC * C], dt.float32,
                           kind="ExternalOutput")

    GB = 14  # blocks per gather/output chunk

    with tile.TileContext(nc) as tc:
        with (
            tc.tile_pool(name="sb", bufs=1) as sb,
            tc.tile_pool(name="rows", bufs=2) as rp_,
        ):
            ep = sb.tile([P, BPC, WU], dt.float32)
            nc.sync.dma_start(
                ep[:], ep_d[:, :].rearrange("p (b w) -> p b w", w=WU))
            sp1 = sb.tile([P, BPC, WU], dt.float32)
            nc.sync.dma_start(
                sp1[:], sp1_d[:, :].rearrange("p (b w) -> p b w", w=WU))

            sm = sb.tile([P, BPC], dt.float32)
            nc.vector.reduce_max(out=sm[:], in_=ep[:],
                                 axis=mybir.AxisListType.X)
            mask = sb.tile([P, BPC, WU], dt.float32)
            sm_b = bass.AP(sm.tensor, 0, [sm[:].ap[0], [1, BPC], [0, WU]])
            nc.vector.tensor_tensor(out=mask[:], in0=ep[:], in1=sm_b,
                                    op=mybir.AluOpType.is_equal)
            cand = sb.tile([P, BPC, WU], dt.float32)
            nc.vector.tensor_tensor(out=cand[:], in0=mask[:], in1=sp1[:],
                                    op=mybir.AluOpType.mult)
            lp1 = sb.tile([P, BPC], dt.float32)
            nc.vector.reduce_max(out=lp1[:], in_=cand[:],
                                 axis=mybir.AxisListType.X)
            leadf = sb.tile([P, BPC], dt.float32)
            nc.vector.tensor_scalar(
                out=leadf[:], in0=lp1[:], scalar1=-1.0, scalar2=0.0,
                op0=mybir.AluOpType.add, op1=mybir.AluOpType.max)
            leadi = sb.tile([P, BPC], dt.int32)
            nc.vector.tensor_copy(leadi[:], leadf[:])

            for g0 in range(0, BPC, GB):
                ng = min(GB, BPC - g0)
                rows = rp_.tile([P, GB * C], dt.float32, tag="rows")
                if batched_indirect:
                    nc.gpsimd.indirect_dma_start(
                        out=rows[:, : ng * C].rearrange(
                            "p (g c) -> p g c", c=C),
                        out_offset=None,
                        in_=rep_d[:, :],
                        in_offset=bass.IndirectOffsetOnAxis(
                            ap=leadi[:, g0: g0 + ng], axis=0),
                    )
                else:
                    for j in range(ng):
                        nc.gpsimd.indirect_dma_start(
                            out=rows[:, j * C: (j + 1) * C],
                            out_offset=None,
                            in_=rep_d[:, :],
                            in_offset=bass.IndirectOffsetOnAxis(
                                ap=leadi[:, g0 + j: g0 + j + 1], axis=0),
                        )
                nc.sync.dma_start(out_o[:, g0 * C: (g0 + ng) * C],
                                  rows[:, : ng * C])
    nc.compile()
    return nc


# ---------------------------------------------------------------------------

_CACHE = {}


def _get(key, fn):
    if key not in _CACHE:
        _CACHE[key] = fn()
    return _CACHE[key]


BATCHED_INDIRECT = False


def kernel(x, edge_index, w_lead, b_lead, w1, b1, w2, b2):
    x = np.asarray(x, np.float32)
    assert x.shape == (N_NODES, H)

    ekey = hashlib.md5(np.asarray(edge_index).tobytes()).hexdigest()
    pp = _get(("pp", ekey), lambda: _preprocess(edge_index))

    NCOL = pp["NCOL"]
    inv = pp["inv"]
    inv_flat = inv.reshape(NCORES, NPC)

    xbf_ext = np.zeros((N_NODES + 1, H), bf16)
    xbf_ext[:N_NODES] = x.astype(bf16)
    xpad = np.zeros((NPAD, H), np.float32)
    xpad[:N_NODES] = x

    wrep = np.tile(np.asarray(w_lead, np.float32)[None, :], (P, 1))
    blead = np.full((P, 1), np.float32(b_lead), np.float32)
    w1f = np.ascontiguousarray(np.asarray(w1, np.float32).astype(bf16))
    b1c = np.ascontiguousarray(np.asarray(b1, np.float32).reshape(H, 1))
    w2f = np.ascontiguousarray(np.asarray(w2, np.float32).astype(bf16))
    b2r = np.ascontiguousarray(
        np.asarray(b2, np.float32).reshape(1, C).astype(bf16))

    # ---- launch 1 ----------------------------------------------------------
    nc1 = _get(("l1", ekey), lambda: _build_l1(pp))
    in_maps = []
    for k in range(NCORES):
        # ET[k]: [P, NCOL*H] partition-major slot table (slot = c*128+p)
        et = xbf_ext[pp["srcidx"][k]].reshape(NCOL, P, H).transpose(1, 0, 2)
        et = np.ascontiguousarray(et).reshape(P, NCOL * H)
        # xf2: [P, BPC*H] partition-major owned-node features
        xf2 = np.where((inv[k] >= 0)[:, :, None],
                       xpad[np.maximum(inv[k], 0)], 0.0)  # [BPC, P, H]
        xf2 = np.ascontiguousarray(
            xf2.transpose(1, 0, 2).reshape(P, BPC * H)).astype(np.float32)
        in_maps.append({
            "et": et,
            "xf2": xf2,
            "dstl": pp["dstl_bf"][k],
            "recip": pp["recip"][k],
            "wrep": wrep,
            "blead": blead,
            "w1": w1f,
            "b1": b1c,
            "w2": w2f,
            "b2": b2r,
        })
    r1 = run_bass_kernel_spmd(nc1, in_maps, core_ids=CORES)

    logits_full = np.zeros(NPAD, np.float32)
    reports_full = np.zeros((NPAD, C), np.float32)
    for k in range(NCORES):
        lg = r1.results[k]["logits_o"]                  # [P, BPC]
        rp = r1.results[k]["reports_o"].reshape(P, BPC, C)
        m = inv_flat[k] >= 0
        ids = inv_flat[k][m]
        logits_full[ids] = lg.T.reshape(-1)[m]
        reports_full[ids] = rp.transpose(1, 0, 2).reshape(NPC, C)[m]

    # ---- launch 2: leader election + gather --------------------------------
    nc2 = _get(("l2", ekey), lambda: _build_l2(pp, BATCHED_INDIRECT))
    es = pp["elog_src"]
    in_maps2 = []
    for k in range(NCORES):
        ep = np.where(es[k] >= 0, logits_full[np.maximum(es[k], 0)], NEG)
        in_maps2.append({
            "epad": np.ascontiguousarray(ep.astype(np.float32)),
            "srcp1": pp["srcp1"][k],
            "repfull": reports_full,
        })
    r2 = run_bass_kernel_spmd(nc2, in_maps2, core_ids=CORES)

    out = np.zeros((N_NODES, C), np.float32)
    for k in range(NCORES):
        g = r2.results[k]["gath_o"].reshape(P, BPC, C)
        node_rows = g.transpose(1, 0, 2).reshape(NPC, C)
        m = inv_flat[k] >= 0
        out[inv_flat[k][m]] = node_rows[m]
    return out
